# revision 8
# baseline (speedup 1.0000x reference)
"""Multi-head attention with RoPE (B=32, N=577, C=768, H=12, D=64) on 8 TRN2 NeuronCores.

Strategy: data-parallel over batch (4 images per core), zero collectives.
Per-core layout: everything channels-on-partitions, tokens-on-free-dim.
  - QKV weights pre-transposed/permuted on host so q,k come out in a
    head-interleaved pair-split layout ([evenA, evenB, oddA, oddB] per
    128-row tile) that makes RoPE a half-partition-swap (legal DVE shift)
    and scores a pair of K=32 row-strip matmuls.
  - v computed in [token, channel] layout (separate matmul orientation)
    with a ones-column appended per head so attn@v also yields the
    softmax denominator (row 64 of each head's output).
  - softmax without max-subtraction (scores*scale ~ N(0,1), |max| ~ 9).
  - matmuls in bf16, softmax/normalization in fp32.
Output computed as [b, c, t] on device; host transposes back.
"""

import sys

sys.path.insert(0, "/opt/trn_rl_repo")

import numpy as np
import ml_dtypes

import concourse.bass as bass
import concourse.bacc as bacc
import concourse.tile as tile
from concourse import mybir
from concourse.bass_utils import run_bass_kernel_spmd

F32 = mybir.dt.float32
BF16 = mybir.dt.bfloat16

B, N, C = 32, 577, 768
H, D = 12, 64
NCORES = 8
BL = B // NCORES  # images per core
SCALE = D ** -0.5
NT = 5  # token tiles: 4*128 + 65
TWS = [128, 128, 128, 128, 65]
# free-dim chunks (psum-bank aligned)
NCH = [(0, 512), (512, 65)]
VCH = [(0, 512), (512, 256)]


def build(n_images=BL, stage="full", norm_mode="dma"):
    nc = bacc.Bacc()
    xT = nc.declare_dram_parameter("xT", [n_images, C, N], BF16, isOutput=False)
    wqk = nc.declare_dram_parameter("wqk", [C, 2 * C], BF16, isOutput=False)
    wv = nc.declare_dram_parameter("wv", [C, C], BF16, isOutput=False)
    wp = nc.declare_dram_parameter("wp", [C, C], BF16, isOutput=False)
    c4d = nc.declare_dram_parameter("c4", [128, N], BF16, isOutput=False)
    s4d = nc.declare_dram_parameter("s4", [128, N], BF16, isOutput=False)
    bpd = nc.declare_dram_parameter("bproj", [6, 128], F32, isOutput=False)
    out = nc.declare_dram_parameter("out", [n_images, C, N], F32, isOutput=True)

    Exp = mybir.ActivationFunctionType.Exp
    MUL = mybir.AluOpType.mult
    ADD = mybir.AluOpType.add

    with tile.TileContext(nc) as tc:
        with (
            tc.tile_pool(name="wpool", bufs=1) as wpool,
            tc.tile_pool(name="xp", bufs=2) as xp,
            tc.tile_pool(name="qkp", bufs=2) as qkp,
            tc.tile_pool(name="vp", bufs=2) as vp,
            tc.tile_pool(name="ep", bufs=2) as ep,
            tc.tile_pool(name="ap", bufs=2) as app,
            tc.tile_pool(name="tp", bufs=3) as tp,
            tc.tile_pool(name="rp", bufs=6) as rp,
            tc.tile_pool(name="op", bufs=3) as op_,
            tc.tile_pool(name="ps", bufs=4, space="PSUM") as ps,
            tc.tile_pool(name="dp", bufs=2, space="DRAM") as dp,
        ):
            # ---- load weights once ----
            wqk_sb = []
            wv_sb = []
            wp_sb = []
            for k in range(6):
                t = wpool.tile([128, 2 * C], BF16, tag=f"wqk{k}")
                nc.sync.dma_start(out=t[:], in_=wqk[k * 128:(k + 1) * 128, :])
                wqk_sb.append(t)
                t = wpool.tile([128, C], BF16, tag=f"wv{k}")
                nc.sync.dma_start(out=t[:], in_=wv[k * 128:(k + 1) * 128, :])
                wv_sb.append(t)
                t = wpool.tile([128, C], BF16, tag=f"wp{k}")
                nc.sync.dma_start(out=t[:], in_=wp[k * 128:(k + 1) * 128, :])
                wp_sb.append(t)
            c4 = wpool.tile([128, N], BF16, tag="c4")
            nc.sync.dma_start(out=c4[:], in_=c4d[:])
            s4 = wpool.tile([128, N], BF16, tag="s4")
            nc.sync.dma_start(out=s4[:], in_=s4d[:])
            bsb = wpool.tile([128, 6], F32, tag="b")
            nc.sync.dma_start(out=bsb[:], in_=bpd[:].transpose([1, 0]))

            for b in range(n_images):
                # ---- stage X: load x transposed ----
                xsb = []
                for k in range(6):
                    t = xp.tile([128, N], BF16, tag=f"x{k}")
                    nc.sync.dma_start(out=t[:], in_=xT[b, k * 128:(k + 1) * 128, :])
                    xsb.append(t)

                # ---- stage QK: qkv matmuls for q,k + RoPE ----
                qk_all = qkp.tile([128, 12, N], BF16, tag="qk")
                for m in range(12):
                    pqk = ps.tile([128, 768], F32, tag="ps")
                    lhs_col = m * 128
                    for k in range(6):
                        lhsT = wqk_sb[k][:, lhs_col:lhs_col + 128]
                        for c0, cw in NCH:
                            nc.tensor.matmul(
                                out=pqk[:, c0:c0 + cw],
                                lhsT=lhsT,
                                rhs=xsb[k][:, c0:c0 + cw],
                                start=(k == 0),
                                stop=(k == 5),
                            )
                    # RoPE: rot = A*C4 + pairswap(A)*S4   (col 0: c=1, s=0)
                    # pairswap via 4 sbuf->sbuf DMAs (32-row blocks), then bf16 DVE ops
                    raw = tp.tile([128, N], BF16, tag="roperaw")
                    nc.vector.tensor_copy(out=raw[:], in_=pqk[:, 0:N])
                    sw = tp.tile([128, N], BF16, tag="ropesw")
                    nc.sync.dma_start(out=sw[0:32, :], in_=raw[32:64, :])
                    nc.sync.dma_start(out=sw[32:64, :], in_=raw[0:32, :])
                    nc.sync.dma_start(out=sw[64:96, :], in_=raw[96:128, :])
                    nc.sync.dma_start(out=sw[96:128, :], in_=raw[64:96, :])
                    tmp = tp.tile([128, N], BF16, tag="ropetmp")
                    rot = tp.tile([128, N], BF16, tag="roperot")
                    nc.vector.tensor_tensor(out=tmp[:], in0=sw[:], in1=s4[:], op=MUL)
                    nc.vector.tensor_tensor(out=rot[:], in0=raw[:], in1=c4[:], op=MUL)
                    nc.vector.tensor_tensor(out=qk_all[:, m, :], in0=rot[:], in1=tmp[:], op=ADD)

                if stage == "qk":
                    continue
                # ---- stage V: v in [token, channel(+ones)] layout ----
                v_all = vp.tile([128, NT, 13 * 65], BF16, tag="v")
                for t_i in range(NT):
                    tw = TWS[t_i]
                    t0 = t_i * 128
                    pv = ps.tile([128, 768], F32, tag="ps")
                    for k in range(6):
                        lhsT = xsb[k][:, t0:t0 + tw]
                        for c0, cw in VCH:
                            nc.tensor.matmul(
                                out=pv[0:tw, c0:c0 + cw],
                                lhsT=lhsT,
                                rhs=wv_sb[k][:, c0:c0 + cw],
                                start=(k == 0),
                                stop=(k == 5),
                            )
                    vdst = v_all[0:tw, t_i, :].rearrange("p (h c) -> p h c", c=65)
                    nc.vector.tensor_copy(
                        out=vdst[:, 0:12, 0:64],
                        in_=pv[0:tw, :].rearrange("p (h d) -> p h d", d=64),
                    )
                    nc.vector.memset(vdst[:, 0:12, 64], 1.0)

                if stage == "v":
                    continue
                # ---- attention per head-pair ----
                attn_all = None
                if stage not in ("scores", "attnv"):
                    attn_all = app.tile([128, 6, N], BF16, tag="attn")
                for m in range(6):
                    qt = qk_all[:, m, :]
                    kt = qk_all[:, 6 + m, :]
                    exps = [
                        ep.tile([128, NT, N], BF16, tag="expA", name="expA"),
                        ep.tile([128, NT, N], BF16, tag="expB", name="expB"),
                    ]
                    for j in range(NT):
                        jw = TWS[j]
                        j0 = j * 128
                        psc = [ps.tile([128, 768], F32, tag="ps", name="pscA"),
                               ps.tile([128, 768], F32, tag="ps", name="pscB")]
                        for c0, cw in NCH:
                            # head 2m at rows 0:64 (row strips 0-1), head 2m+1 at 64:128 (strips 2-3)
                            nc.tensor.matmul(out=psc[0][0:jw, c0:c0 + cw], lhsT=kt[0:64, j0:j0 + jw],
                                             rhs=qt[0:64, c0:c0 + cw], start=True, stop=True)
                            nc.tensor.matmul(out=psc[1][0:jw, c0:c0 + cw], lhsT=kt[64:128, j0:j0 + jw],
                                             rhs=qt[64:128, c0:c0 + cw], start=True, stop=True)
                        for hh in range(2):
                            nc.scalar.activation(out=exps[hh][0:jw, j, :], in_=psc[hh][0:jw, 0:N],
                                                 func=Exp, scale=SCALE)

                    if stage == "scores":
                        continue
                    for hh in range(2):
                        h = 2 * m + hh
                        po = ps.tile([128, 768], F32, tag="ps")
                        for j in range(NT):
                            jw = TWS[j]
                            lhsT = v_all[0:jw, j, :].rearrange("p (h c) -> p h c", c=65)[:, h, :]
                            for c0, cw in NCH:
                                nc.tensor.matmul(
                                    out=po[0:65, c0:c0 + cw],
                                    lhsT=lhsT,
                                    rhs=exps[hh][0:jw, j, c0:c0 + cw],
                                    start=(j == 0),
                                    stop=(j == NT - 1),
                                )
                        if stage == "attnv":
                            continue
                        # denominators: row 64 = sum_j exp
                        r1 = rp.tile([1, N], F32, tag="rimg")
                        nc.vector.reciprocal(out=r1[:], in_=po[64:65, 0:N])
                        if norm_mode == "dma":
                            drt = dp.tile([1, N], F32, tag="rscr")
                            nc.sync.dma_start(out=drt[:], in_=r1[:])
                            rbc = rp.tile([64, N], F32, tag="rbc")
                            nc.sync.dma_start(out=rbc[:], in_=drt[0, :].partition_broadcast(64))
                            nc.vector.tensor_tensor(
                                out=attn_all[hh * 64:(hh + 1) * 64, m, :],
                                in0=po[0:64, 0:N],
                                in1=rbc[:],
                                op=MUL,
                            )
                        else:
                            nc.vector.tensor_copy(
                                out=attn_all[hh * 64:(hh + 1) * 64, m, :],
                                in_=po[0:64, 0:N],
                            )

                if stage in ("attn", "scores", "attnv"):
                    continue
                # ---- proj ----
                for ct in range(6):
                    pp = ps.tile([128, 768], F32, tag="ps")
                    for k in range(6):
                        lhsT = wp_sb[k][:, ct * 128:(ct + 1) * 128]
                        for c0, cw in NCH:
                            nc.tensor.matmul(
                                out=pp[:, c0:c0 + cw],
                                lhsT=lhsT,
                                rhs=attn_all[:, k, c0:c0 + cw],
                                start=(k == 0),
                                stop=(k == 5),
                            )
                    osb = op_.tile([128, N], F32, tag="osb")
                    nc.vector.tensor_scalar_add(out=osb[:], in0=pp[:, 0:N], scalar1=bsb[:, ct:ct + 1])
                    nc.sync.dma_start(out=out[b, ct * 128:(ct + 1) * 128, :], in_=osb[:])
    nc.compile()
    return nc


def _qk_perm():
    """Row permutation of w_qkv's q,k sections -> head-interleaved pair-split."""
    perm = np.zeros(2 * C, dtype=np.int64)
    for m in range(12):
        sec = 0 if m < 6 else 1
        pair = m % 6
        base = m * 128
        hA, hB = 2 * pair, 2 * pair + 1
        perm[base + 0:base + 32] = sec * C + hA * D + 2 * np.arange(32)
        perm[base + 32:base + 64] = sec * C + hA * D + 2 * np.arange(32) + 1
        perm[base + 64:base + 96] = sec * C + hB * D + 2 * np.arange(32)
        perm[base + 96:base + 128] = sec * C + hB * D + 2 * np.arange(32) + 1
    return perm


def prep_inputs(x, w_qkv, w_proj, b_proj, cos, sin, n_images=BL):
    bf16 = ml_dtypes.bfloat16
    perm = _qk_perm()
    wqk = np.ascontiguousarray(w_qkv[perm, :].T).astype(bf16)  # [C, 2C]
    wv = np.ascontiguousarray(w_qkv[2 * C:3 * C, :].T).astype(bf16)  # [C, C]
    wp = np.ascontiguousarray(w_proj.T).astype(bf16)  # [C(in), C(out)]

    c4 = np.ones((128, N), dtype=np.float32)
    s4 = np.zeros((128, N), dtype=np.float32)
    p = np.arange(128)
    c4[:, 1:] = cos[:, p % 32].T
    s4[:, 1:] = sin[:, p % 32].T * np.where((p // 32) % 2 == 0, -1.0, 1.0)[:, None]
    c4 = c4.astype(bf16)
    s4 = s4.astype(bf16)

    bp = np.ascontiguousarray(b_proj.reshape(6, 128)).astype(np.float32)

    xT = np.ascontiguousarray(np.transpose(x, (0, 2, 1))).astype(bf16)  # [B, C, N]

    in_maps = []
    for i in range(NCORES):
        in_maps.append({
            "xT": xT[i * n_images:(i + 1) * n_images],
            "wqk": wqk, "wv": wv, "wp": wp,
            "c4": c4, "s4": s4, "bproj": bp,
        })
    return in_maps


_BUILT = {}


def kernel(x, w_qkv, w_proj, b_proj, cos, sin):
    x = np.asarray(x, dtype=np.float32)
    w_qkv = np.asarray(w_qkv, dtype=np.float32)
    w_proj = np.asarray(w_proj, dtype=np.float32)
    b_proj = np.asarray(b_proj, dtype=np.float32)
    cos = np.asarray(cos, dtype=np.float32)
    sin = np.asarray(sin, dtype=np.float32)

    if "nc" not in _BUILT:
        _BUILT["nc"] = build()
    nc = _BUILT["nc"]
    in_maps = prep_inputs(x, w_qkv, w_proj, b_proj, cos, sin)
    res = run_bass_kernel_spmd(nc, in_maps, core_ids=list(range(NCORES)))
    outs = np.concatenate([np.asarray(res.results[i]["out"]) for i in range(NCORES)], axis=0)
    return np.ascontiguousarray(np.transpose(outs, (0, 2, 1))).astype(np.float32)
